# revision 1
# baseline (speedup 1.0000x reference)
"""CTC loss kernel for Trainium2, data-parallel over batch across 8 NeuronCores.

Problem: pred [64, 64, 6736] f32 logits, gt [64, 16] int labels (< blank).
loss = mean_n( -log p_ctc(gt_n | log_softmax(pred_n)) / S ).

Per-core plan (8 examples/core):
  - Stream pred shard (13.8 MB) through SBUF in [128 = 8 ex x 16 t, C] chunks.
    ACT exp + accum_out gives s[n,t] = sum_c exp(pred[n,t,c]) without
    materializing log_softmax (no max subtraction needed: |logits| ~ 5).
  - Gather the 16 target-label logits per (n, t) with per-partition
    indirect DMAs (HW semantics: one offset per partition, consecutive row;
    we use rows of one element, one call per (t-block, label j)).  The
    blank column is a static strided DMA.
  - CTC forward DP in *unnormalized* prob domain on DVE: alphaU *= u_t where
    u_t = exp(pred_ext).  Rescale every few steps (sum / reciprocal / mul),
    collecting the rescale factors; logs deferred to one ACT Ln pass at the
    end so the ACT exp/ln tables swap only once.
  - log p = ln(alphaU[L-1]+alphaU[L-2]) + sum(ln C_k) - sum_t ln s[n,t].
  - Core writes per-example nll/S ([8,1]); host concatenates and means.

Hardware quirks handled here:
  - any instruction fits only ONE sync-wait command: _split_multi_waits
    legalizes every multi-wait instruction into a chain of single-wait
    no-ops on the same engine; HWDGE DMA count is kept <= 8 and consecutive
    ACT ops get disjoint tiles to keep the wait count low in the first
    place.
"""

import os

import numpy as np

# Persistent XLA compilation cache: makes repeat kernel() calls skip the
# multi-minute neuronx-cc compile when the program is unchanged.
os.environ.setdefault("JAX_COMPILATION_CACHE_DIR", "/tmp/jax_comp_cache")

import concourse.bass as bass
import concourse.mybir as mybir
import concourse.tile as tile
from concourse.bass_utils import run_bass_kernel_spmd

F32 = mybir.dt.float32
I32 = mybir.dt.int32
AF = mybir.ActivationFunctionType
ALU = mybir.AluOpType

# Problem constants
N, T, C, S = 64, 64, 6736, 16
BLANK = C - 1
NCORES = 8
NL = N // NCORES            # examples per core
L = 2 * S + 1               # 33 extended labels
LP = L + 1                  # padded to 34 (dummy slot stays 0)
NTB = 4                     # t-blocks of 16 timesteps -> 128 partitions
TB = T // NTB
CH = C // 2
RENORM_EVERY = 8
RENORM_TS = [t for t in range(1, T) if t % RENORM_EVERY == 0]
NRE = len(RENORM_TS)        # 10

NOFF = NTB * S              # 64 offset columns (tb-major, then j)
AUXW = NOFF + LP            # + 34 bitcast mask columns (rows 0..NL)


def build_bass():
    nc = bass.Bass()
    pred = nc.dram_tensor("pred", [NL, T, C], F32, kind="ExternalInput")
    aux = nc.dram_tensor("aux", [128, AUXW], I32, kind="ExternalInput")
    out = nc.dram_tensor("out", [NL, 1], F32, kind="ExternalOutput")

    pred_flat = pred[:].rearrange("n t c -> (n t c)").unsqueeze(-1)

    with tile.TileContext(nc) as tc:
        with (
            tc.tile_pool(name="stream", bufs=1) as bp,
            tc.tile_pool(name="small", bufs=1) as sp,
        ):
            aux_t = sp.tile([128, AUXW], I32)
            nc.sync.dma_start(out=aux_t[:], in_=aux[:])
            mask_t = aux_t[0:NL, NOFF:AUXW].bitcast(F32)

            # blank column: static strided DMA + exp (SWDGE: HWDGE lanes are
            # reserved for the big transfers, and Pool's queue is idle early)
            blankT = sp.tile([NL, T], F32)
            nc.gpsimd.dma_start(out=blankT[:], in_=pred[:, :, BLANK : BLANK + 1])
            blankE = sp.tile([NL, T], F32)
            nc.scalar.activation(blankE[:], blankT[:], AF.Exp)

            # u[n, t, l]: even l -> blank, odd l < 2S -> target, l=LP-1 -> 0
            u = sp.tile([NL, T * LP], F32)
            u3 = u[:].rearrange("n (t l) -> n t l", l=LP)
            nc.vector.memset(u3[:, :, LP - 1], 0.0)
            blank_bcast = bass.AP(
                blankE.tensor, blankE[:].offset, [blankE[:].ap[0], [1, T], [0, S + 1]]
            )
            nc.vector.tensor_copy(out=u3[:, :, 0 : 2 * S + 1 : 2], in_=blank_bcast)

            # ---------------- interleaved gather / stream / DP -------------
            # Engines execute in program order, so emission order is the
            # schedule: gather block tb's exp must precede later stream-chunk
            # exps on ACT, DP block tb (DVE) follows its u-copy, and the
            # stile copies sit after the whole DP.  Stream DMAs go on the SP
            # ring right after aux so streaming starts immediately.
            buf = sp.tile([NL, LP + 2], F32)  # cols 0,1 guard zeros; 2.. = alpha
            nc.vector.memset(buf[:], 0.0)
            tmp1 = sp.tile([NL, LP], F32)
            tmp2 = sp.tile([NL, LP], F32)
            tmp3 = sp.tile([NL, LP], F32)
            rlog = sp.tile([NL, NRE + 1], F32)
            rinv = sp.tile([NL, 1], F32)
            a = buf[:, 2 : LP + 2]
            a1 = buf[:, 1 : LP + 1]
            a2 = buf[:, 0:LP]

            chunks = [(k, 0, C) for k in range(NTB - 1)]
            chunks += [(NTB - 1, 0, CH), (NTB - 1, CH, CH)]
            hparts = {}

            # Issue every stream DMA up front on the SP HWDGE ring (the ACT
            # ring hard-faults this device), so streaming starts immediately
            # and is decoupled from ACT's in-order exp schedule.
            stream_tiles = []
            for ci, (k, c0, cw) in enumerate(chunks):
                bt = bp.tile([128, cw], F32, tag=f"stream{ci}")
                src = pred[:, k * TB : (k + 1) * TB, c0 : c0 + cw]
                nc.sync.dma_start(out=bt[:], in_=src)
                stream_tiles.append(bt)

            def emit_chunk(ci):
                k, c0, cw = chunks[ci]
                bt = stream_tiles[ci]
                hk = sp.tile([128, 1], F32, tag=f"h{ci}")
                nc.scalar.activation(bt[:], bt[:], AF.Exp, accum_out=hk[:])
                hparts.setdefault(k, []).append(hk)

            def emit_gather(tb):
                pg2 = sp.tile([128, S], F32, tag=f"pg2_{tb}")
                for j in range(S):
                    col = tb * S + j
                    nc.gpsimd.indirect_dma_start(
                        out=pg2[:, j : j + 1],
                        out_offset=None,
                        in_=pred_flat,
                        in_offset=bass.IndirectOffsetOnAxis(
                            ap=aux_t[:, col : col + 1], axis=0
                        ),
                    )
                pg3 = sp.tile([NL, TB * S], F32, tag=f"pg3_{tb}")
                # [128=(n,tt), j] -> [n, (tt, j)]
                nc.gpsimd.dma_start(
                    out=pg3[:].rearrange("n (tt j) -> n tt j", j=S), in_=pg2[:]
                )
                ug = sp.tile([NL, TB * S], F32, tag=f"ug_{tb}")
                nc.scalar.activation(ug[:], pg3[:], AF.Exp)
                return ug

            def emit_dp_block(tb, ug):
                ug3 = ug[:].rearrange("n (tt j) -> n tt j", j=S)
                nc.vector.tensor_copy(
                    out=u3[:, tb * TB : (tb + 1) * TB, 1 : 2 * S : 2], in_=ug3
                )
                if tb == 0:
                    nc.vector.tensor_copy(out=buf[:, 2:4], in_=u3[:, 0, 0:2])
                for t in range(max(1, tb * TB), (tb + 1) * TB):
                    ut = u3[:, t, :]
                    nc.vector.tensor_add(out=tmp1[:], in0=a, in1=a1)
                    nc.vector.tensor_mul(out=tmp2[:], in0=a2, in1=mask_t)
                    nc.vector.tensor_add(out=tmp3[:], in0=tmp1[:], in1=tmp2[:])
                    nc.vector.tensor_mul(out=a, in0=tmp3[:], in1=ut)
                    if t in RENORM_TS:
                        k = RENORM_TS.index(t)
                        nc.vector.tensor_reduce(
                            out=rlog[:, k : k + 1],
                            in_=a,
                            axis=mybir.AxisListType.X,
                            op=ALU.add,
                        )
                        nc.vector.reciprocal(out=rinv[:], in_=rlog[:, k : k + 1])
                        nc.vector.tensor_scalar_mul(out=a, in0=a, scalar1=rinv[:])

            ug0 = emit_gather(0)
            emit_chunk(0)
            emit_dp_block(0, ug0)
            ug1 = emit_gather(1)
            emit_dp_block(1, ug1)
            ug2 = emit_gather(2)
            emit_chunk(1)
            emit_dp_block(2, ug2)
            ug3b = emit_gather(3)
            emit_chunk(2)
            emit_dp_block(3, ug3b)
            emit_chunk(3)
            emit_chunk(4)

            stile = sp.tile([128, NTB], F32)
            for k in range(NTB):
                hs = hparts[k]
                if len(hs) == 1:
                    nc.vector.tensor_copy(out=stile[:, k : k + 1], in_=hs[0][:])
                else:
                    nc.vector.tensor_add(
                        out=stile[:, k : k + 1], in0=hs[0][:], in1=hs[1][:]
                    )

            # final forward prob: alpha[L-1] + alpha[L-2] (cols L+1, L in buf)
            nc.vector.tensor_add(
                out=rlog[:, NRE : NRE + 1],
                in0=buf[:, L : L + 1],
                in1=buf[:, L + 1 : L + 2],
            )

            # ---------------- logs + assembly ------------------------------
            lnr = sp.tile([NL, NRE + 1], F32)
            nc.scalar.activation(lnr[:], rlog[:], AF.Ln)
            rsum = sp.tile([NL, 1], F32)
            nc.vector.tensor_reduce(
                out=rsum[:], in_=lnr[:], axis=mybir.AxisListType.X, op=ALU.add
            )

            lns = sp.tile([128, NTB], F32)
            nc.scalar.activation(lns[:], stile[:], AF.Ln)
            sb = sp.tile([128, 1], F32)
            nc.vector.tensor_reduce(
                out=sb[:], in_=lns[:], axis=mybir.AxisListType.X, op=ALU.add
            )
            # regroup partitions (n*TB + tt) -> [NL, TB] and finish the sum
            zt = sp.tile([NL, TB], F32)
            nc.sync.dma_start(out=zt[:], in_=sb[:])
            zs = sp.tile([NL, 1], F32)
            nc.vector.tensor_reduce(
                out=zs[:], in_=zt[:], axis=mybir.AxisListType.X, op=ALU.add
            )

            # nll/S = (sum_t ln s - (ln fwd + sum ln C_k)) / S
            res = sp.tile([NL, 1], F32)
            nc.vector.tensor_tensor(
                out=res[:], in0=zs[:], in1=rsum[:], op=ALU.subtract
            )
            res2 = sp.tile([NL, 1], F32)
            nc.vector.tensor_scalar_mul(out=res2[:], in0=res[:], scalar1=1.0 / S)
            nc.sync.dma_start(out=out[:], in_=res2[:])

    return nc


def _split_multi_waits(nc, maxw=1):
    """This compiler's codegen rejects >1 sync-wait command per instruction
    (setupSyncWait 'Too many sync wait commands').  Tile's kernel-tail drain
    aggregates one wait per live semaphore; split the excess into a chain of
    single-wait no-ops on the same engine right before the instruction."""
    for bb in nc.main_func.blocks:
        heavy = [
            (i, inst)
            for i, inst in enumerate(bb.instructions)
            if getattr(inst, "sync_info", None) is not None
            and inst.sync_info.on_wait
            and len(inst.sync_info.on_wait) > maxw
        ]
        for pos, inst in reversed(heavy):
            waits = list(inst.sync_info.on_wait)
            keep, extra = waits[:maxw], waits[maxw:]
            inst.sync_info = mybir.SyncInfo(
                on_wait=keep, on_update=list(inst.sync_info.on_update)
            )
            for j, w in enumerate(reversed(extra)):
                nop = mybir.InstNoOp(
                    name=f"{inst.name}-waitsplit-{j}",
                    ins=[],
                    outs=[],
                    sync_info=mybir.SyncInfo(on_wait=[w], on_update=[]),
                )
                nop.engine = inst.engine
                bb.instructions.insert(pos, nop)


def prepare_hw(nc):
    _split_multi_waits(nc)
    return nc


def make_core_inputs(pred_full, gt_full, core):
    nsl = slice(core * NL, (core + 1) * NL)
    predc = np.ascontiguousarray(pred_full[nsl], dtype=np.float32)
    gtc = np.asarray(gt_full[nsl]).astype(np.int64)

    # offsets: col = tb*S + j; partition p = (n, tt):
    #   flat elem index of pred[n, 16*tb + tt, gt[n, j]]
    aux = np.zeros((128, AUXW), np.int32)
    p_n = np.arange(128) // TB
    p_tt = np.arange(128) % TB
    for tb in range(NTB):
        t_abs = tb * TB + p_tt  # [128]
        base = (p_n * T + t_abs) * C
        for j in range(S):
            aux[:, tb * S + j] = base + gtc[p_n, j]

    m = np.zeros((NL, LP), np.float32)
    m[:, 1] = 1.0
    for j in range(1, S):
        m[:, 2 * j + 1] = (gtc[:, j] != gtc[:, j - 1]).astype(np.float32)
    aux[0:NL, NOFF:AUXW] = m.view(np.int32)

    return {"pred": predc, "aux": aux}


_NC_CACHE = {}


def kernel(pred, gt):
    in_maps = [make_core_inputs(pred, gt, c) for c in range(NCORES)]
    if "nc" not in _NC_CACHE:
        _NC_CACHE["nc"] = prepare_hw(build_bass())
    nc = _NC_CACHE["nc"]
    res = run_bass_kernel_spmd(nc, in_maps, core_ids=list(range(NCORES)))
    _NC_CACHE["last_results"] = res
    vals = np.concatenate([r["out"][:, 0] for r in res.results])
    return np.array(vals.mean(), dtype=np.float32)


if __name__ == "__main__":
    rng = np.random.default_rng(0)
    pred = rng.standard_normal((N, T, C), dtype=np.float32)
    gt = rng.integers(0, BLANK, size=(N, S)).astype(np.int32)
    print(kernel(pred=pred, gt=gt))



# revision 4
# speedup vs baseline: 3.0330x; 3.0330x over previous
"""CTC loss kernel for Trainium2, data-parallel over batch across 8 NeuronCores.

Problem: pred [64, 64, 6736] f32 logits, gt [64, 16] int labels (< blank).
loss = mean_n( -log p_ctc(gt_n | log_softmax(pred_n)) / S ).

Per-core plan (8 examples/core), v2 — c-major layout:
  - Host stages the core's pred shard transposed to c-major bf16
    [C_pad=6784, n=8, t=64] (6.9 MB).  In this layout every (c, n) pair
    owns a contiguous 64-element t-row, so the label gather is ONE
    indirect DMA of 128 rows (vs 64 tiny per-label gathers before), and
    the big stream DMA still moves 1KB-contiguous lines per partition.
  - Stream tiles are [128 = c-block, 512 = (n t)].  ACT exp (bf16 out)
    feeds a ones-vector matmul on the otherwise idle PE that accumulates
    the per-(n,t) softmax denominators s[n,t] across the 53 c-blocks in
    one PSUM bank (cross-partition reduction).
  - CTC forward DP on DVE in the unnormalized prob domain, 3 ops/step:
    the skip-transition mask is structural (blank-interleaved, no
    adjacent label repeats in the data) so the l-2 term is a strided
    16-wide add; rescaling every 8 steps costs only a reciprocal — the
    rescale multiply is fused into the next step via scalar_tensor_tensor
    and the rescale sums come free from tensor_tensor_reduce.
  - log p = ln(alpha[L-1]+alpha[L-2]) + sum ln C_k - sum_t ln s[n,t];
    per-example nll/S written to dram; host concatenates and means.

Hardware quirks handled here:
  - any instruction fits only ONE sync-wait command: _split_multi_waits
    legalizes every multi-wait instruction into a chain of single-wait
    no-ops on the same engine.
  - the ACT HWDGE ring hard-faults this device: all direct DMAs go on
    the SP ring, indirect/regroup on the gpsimd SWDGE ring.
"""

import os

import numpy as np
import ml_dtypes

# Persistent XLA compilation cache: makes repeat kernel() calls skip the
# multi-minute neuronx-cc compile when the program is unchanged.
os.environ.setdefault("JAX_COMPILATION_CACHE_DIR", "/tmp/jax_comp_cache")

import concourse.bass as bass
import concourse.mybir as mybir
import concourse.tile as tile
from concourse.bass_utils import run_bass_kernel_spmd

F32 = mybir.dt.float32
BF16 = mybir.dt.bfloat16
I32 = mybir.dt.int32
AF = mybir.ActivationFunctionType
ALU = mybir.AluOpType

# Problem constants
N, T, C, S = 64, 64, 6736, 16
BLANK = C - 1
NCORES = 8
NL = N // NCORES            # examples per core
L = 2 * S + 1               # 33 extended labels
LP = L + 1                  # 34 (pad)
CPAD = 6784                 # 53 * 128
NCB = CPAD // 128           # 53 c-blocks
ROWS = CPAD * NL            # 54272 rows of T elements in the c-major view
SFREE = NL * T              # 512 (n, t) columns per c-block
CHUNKS = [8, 13, 16, 16]    # c-blocks per stream DMA (front-loaded small)
RENORM_TS = [t for t in range(1, T) if t % 8 == 0]
NRE = len(RENORM_TS)        # 7
PAD_VAL = -30000.0          # exp() underflows to exactly 0

AUXW = 1 + LP               # col 0: gather row idx; cols 1..34: mask (f32 bits)


def build_bass(structural: bool):
    nc = bass.Bass()
    predT = nc.dram_tensor("predT", [ROWS, T], BF16, kind="ExternalInput")
    aux = nc.dram_tensor("aux", [128, AUXW], I32, kind="ExternalInput")
    out = nc.dram_tensor("out", [NL, 1], F32, kind="ExternalOutput")

    with tile.TileContext(nc) as tc:
        with (
            tc.tile_pool(name="big", bufs=1) as bp,
            tc.tile_pool(name="small", bufs=1) as sp,
            tc.tile_pool(name="ps", bufs=1, space="PSUM") as pp,
        ):
            # ---------- small loads first on the SP ring ----------
            aux_t = sp.tile([128, AUXW], I32)
            nc.sync.dma_start(out=aux_t[:], in_=aux[:])
            blk = sp.tile([NL, T], BF16)
            nc.sync.dma_start(
                out=blk[:], in_=predT[BLANK * NL : BLANK * NL + NL, :]
            )

            # ---------- big stream DMAs, issued up-front ----------
            view = predT[:].rearrange("(b p n) t -> b p (n t)", p=128, n=NL)
            raw_tiles = []
            col = 0
            for ci, ch in enumerate(CHUNKS):
                rt = bp.tile([128, ch * SFREE], BF16, tag=f"raw{ci}")
                nc.sync.dma_start(
                    out=rt[:].rearrange("p (b f) -> p b f", f=SFREE),
                    in_=view[col : col + ch].rearrange("b p f -> p b f"),
                )
                raw_tiles.append(rt)
                col += ch

            # ---------- label gather: ONE indirect DMA ----------
            pg = sp.tile([128, T], BF16)  # partition (n, j), row = t
            nc.gpsimd.indirect_dma_start(
                out=pg[:],
                out_offset=None,
                in_=predT[:],
                in_offset=bass.IndirectOffsetOnAxis(ap=aux_t[:, 0:1], axis=0),
            )
            # regroup [128=(n,j), t] -> [8, (j t)] (same linear order)
            pgr = sp.tile([NL, S * T], BF16)
            nc.gpsimd.dma_start(out=pgr[:], in_=pg[:])

            # ---------- u3[n, t, l] built by ACT exp directly ----------
            u3 = sp.tile([NL, T * LP], F32)
            u3v = u3[:].rearrange("n (t l) -> n t l", l=LP)
            pgr_v = pgr[:].rearrange("n (j t) -> n t j", t=T)
            nc.scalar.activation(u3v[:, :, 1 : 2 * S : 2], pgr_v, AF.Exp)
            blk_b = bass.AP(
                blk.tensor, blk[:].offset, [blk[:].ap[0], [1, T], [0, S + 1]]
            )
            nc.scalar.activation(u3v[:, :, 0 : 2 * S + 1 : 2], blk_b, AF.Exp)

            # ---------- stream exp (bf16 out) ----------
            exp_tiles = []
            for ci, ch in enumerate(CHUNKS):
                et = bp.tile([128, ch * SFREE], BF16, tag=f"exp{ci}")
                nc.scalar.activation(et[:], raw_tiles[ci][:], AF.Exp)
                exp_tiles.append(et)

            # ---------- denominator: ones-matmul accumulation ----------
            ones = sp.tile([128, 1], BF16)
            nc.vector.memset(ones[:], 1.0)
            psum = pp.tile([1, SFREE], F32)
            ki = 0
            for ci, ch in enumerate(CHUNKS):
                for b in range(ch):
                    nc.tensor.matmul(
                        psum[:],
                        ones[:],
                        exp_tiles[ci][:, b * SFREE : (b + 1) * SFREE],
                        start=(ki == 0),
                        stop=(ki == NCB - 1),
                    )
                    ki += 1

            # ---------- CTC forward DP on DVE ----------
            buf = sp.tile([NL, LP + 1], F32)  # cols 0,1 guard zeros; 2.. alpha
            nc.vector.memset(buf[:], 0.0)
            x = sp.tile([NL, L], F32)
            x2 = sp.tile([NL, L], F32)
            rlog = sp.tile([NL, NRE + 1], F32)
            rinv = sp.tile([NL, 1], F32)
            a = buf[:, 2 : 2 + L]
            a1 = buf[:, 1 : 1 + L]
            a2 = buf[:, 0:L]
            mask_t = aux_t[0:NL, 1 : 1 + L].bitcast(F32)

            nc.vector.tensor_copy(out=buf[:, 2:4], in_=u3v[:, 0, 0:2])
            pending = False
            for t in range(1, T):
                ut = u3v[:, t, 0:L]
                nc.vector.tensor_add(out=x[:], in0=a, in1=a1)
                if structural:
                    # skip term lands on odd l only; a2|odd = buf cols 1,3..31
                    nc.vector.tensor_add(
                        out=x[:, 1:L:2], in0=x[:, 1:L:2], in1=buf[:, 1:32:2]
                    )
                else:
                    nc.vector.tensor_mul(out=x2[:], in0=a2, in1=mask_t)
                    nc.vector.tensor_add(out=x[:], in0=x[:], in1=x2[:])
                if pending:
                    # fused rescale from the renorm one step ago
                    nc.vector.scalar_tensor_tensor(
                        out=a,
                        in0=x[:],
                        scalar=rinv[:, 0:1],
                        in1=ut,
                        op0=ALU.mult,
                        op1=ALU.mult,
                    )
                    pending = False
                elif t in RENORM_TS:
                    k = RENORM_TS.index(t)
                    nc.vector.tensor_mul(out=a, in0=x[:], in1=ut)
                    nc.vector.tensor_reduce(
                        out=rlog[:, k : k + 1],
                        in_=a,
                        axis=mybir.AxisListType.X,
                        op=ALU.add,
                    )
                    nc.vector.reciprocal(out=rinv[:], in_=rlog[:, k : k + 1])
                    pending = True
                else:
                    nc.vector.tensor_mul(out=a, in0=x[:], in1=ut)
            # final forward prob: alpha[L-1] + alpha[L-2] = buf cols 34, 33
            nc.vector.tensor_add(
                out=rlog[:, NRE : NRE + 1],
                in0=buf[:, 33:34],
                in1=buf[:, 34:35],
            )

            # ---------- s path: ln + regroup + reduce ----------
            lns = sp.tile([1, SFREE], F32)
            nc.scalar.activation(lns[:], psum[:], AF.Ln)
            srg = sp.tile([NL, T], F32)
            nc.sync.dma_start(out=srg[:], in_=lns[:])
            ssum = sp.tile([NL, 1], F32)
            nc.vector.tensor_reduce(
                out=ssum[:], in_=srg[:], axis=mybir.AxisListType.X, op=ALU.add
            )

            # ---------- assembly ----------
            lnr = sp.tile([NL, NRE + 1], F32)
            nc.scalar.activation(lnr[:], rlog[:], AF.Ln)
            rsum = sp.tile([NL, 1], F32)
            nc.vector.tensor_reduce(
                out=rsum[:], in_=lnr[:], axis=mybir.AxisListType.X, op=ALU.add
            )
            res = sp.tile([NL, 1], F32)
            nc.vector.tensor_tensor(
                out=res[:], in0=ssum[:], in1=rsum[:], op=ALU.subtract
            )
            res2 = sp.tile([NL, 1], F32)
            nc.vector.tensor_scalar_mul(out=res2[:], in0=res[:], scalar1=1.0 / S)
            nc.sync.dma_start(out=out[:], in_=res2[:])

    return nc


def _split_multi_waits(nc, maxw=1):
    """This compiler's codegen rejects >1 sync-wait command per instruction
    (setupSyncWait 'Too many sync wait commands').  Tile's kernel-tail drain
    aggregates one wait per live semaphore; split the excess into a chain of
    single-wait no-ops on the same engine right before the instruction."""
    for bb in nc.main_func.blocks:
        heavy = [
            (i, inst)
            for i, inst in enumerate(bb.instructions)
            if getattr(inst, "sync_info", None) is not None
            and inst.sync_info.on_wait
            and len(inst.sync_info.on_wait) > maxw
        ]
        for pos, inst in reversed(heavy):
            waits = list(inst.sync_info.on_wait)
            keep, extra = waits[:maxw], waits[maxw:]
            inst.sync_info = mybir.SyncInfo(
                on_wait=keep, on_update=list(inst.sync_info.on_update)
            )
            for j, w in enumerate(reversed(extra)):
                nop = mybir.InstNoOp(
                    name=f"{inst.name}-waitsplit-{j}",
                    ins=[],
                    outs=[],
                    sync_info=mybir.SyncInfo(on_wait=[w], on_update=[]),
                )
                nop.engine = inst.engine
                bb.instructions.insert(pos, nop)


def prepare_hw(nc):
    _split_multi_waits(nc)
    return nc


def make_core_inputs(pred_full, gt_full, core):
    nsl = slice(core * NL, (core + 1) * NL)
    predc = np.asarray(pred_full[nsl], dtype=np.float32)  # [8, 64, 6736]
    gtc = np.asarray(gt_full[nsl]).astype(np.int64)

    # c-major bf16: [CPAD, NL, T] -> [ROWS, T]
    pt = np.full((CPAD, NL, T), PAD_VAL, dtype=np.float32)
    pt[:C] = predc.transpose(2, 0, 1)
    predT = pt.reshape(ROWS, T).astype(ml_dtypes.bfloat16)

    aux = np.zeros((128, AUXW), np.int32)
    # gather row index for partition q = (n, j): row = gt[n, j] * NL + n
    p_n = np.arange(128) // S
    p_j = np.arange(128) % S
    aux[:, 0] = gtc[p_n, p_j] * NL + p_n

    m = np.zeros((NL, LP), np.float32)
    m[:, 1] = 1.0
    for j in range(1, S):
        m[:, 2 * j + 1] = (gtc[:, j] != gtc[:, j - 1]).astype(np.float32)
    aux[0:NL, 1 : 1 + LP] = m.view(np.int32)

    return {"predT": predT, "aux": aux}


_NC_CACHE = {}


def kernel(pred, gt):
    gt64 = np.asarray(gt).astype(np.int64)
    structural = not bool((gt64[:, 1:] == gt64[:, :-1]).any())
    in_maps = [make_core_inputs(pred, gt, c) for c in range(NCORES)]
    key = f"nc_{structural}"
    if key not in _NC_CACHE:
        _NC_CACHE[key] = prepare_hw(build_bass(structural))
    nc = _NC_CACHE[key]
    res = run_bass_kernel_spmd(nc, in_maps, core_ids=list(range(NCORES)))
    _NC_CACHE["last_results"] = res
    vals = np.concatenate([r["out"][:, 0] for r in res.results])
    return np.array(vals.mean(), dtype=np.float32)


if __name__ == "__main__":
    rng = np.random.default_rng(0)
    pred = rng.standard_normal((N, T, C), dtype=np.float32)
    gt = rng.integers(0, BLANK, size=(N, S)).astype(np.int32)
    print(kernel(pred=pred, gt=gt))
